# revision 29
# baseline (speedup 1.0000x reference)
"""Causal single-head attention (S=8192, d=64) on 8 Trainium2 NeuronCores.

Strategy (sequence-parallel, load-balanced over the causal triangle):
  - Split the sequence into 16 chunks of 512 rows. Core m owns query chunks
    A=m and B=15-m, so every core sees (m+1) + (16-m) = 17 (kv-block, q-chunk)
    pairs of 512x512 scores -- perfectly balanced.
  - Each pair is one "slot". The SPMD program is identical on all cores; the
    per-core schedule is baked into the *input data* (host gathers the slot's
    kv rows and q rows, transposed and bias-augmented, in fp16).
  - Per slot: qtil = M @ xqT (M = wk_aug wq_aug^T / 8, host-precomputed from
    weights), scores sT[j, i] = xkvT^T @ qtil (kv j on partitions), pT =
    exp(sT).
  - The V projection is reassociated out of the slot loop entirely:
      P @ (Xk wv^T + bv) = wv_aug^T @ (Xk_aug^T @ P^T)
    so each slot only computes z = x_rows^T @ pT (x_rows = the kv rows in
    row-major, bias-augmented; row 64 of z = softmax denominator), and wv_aug
    is applied once per q chunk at the end, fused into the PE transpose that
    produces the output layout.
  - The causal mask on the two diagonal slots is handled by computing only the
    causal trapezoid (packed into PSUM) + post-exp affine_select on the four
    128x128 diagonal blocks, exactly matching softmax(scores - 1e10*mask).
  - Slots are ordered [diag-A, A-pairs, B-pairs, diag-B]. Slots 0..8 (the
    data-dependent A/B mix) accumulate z into SBUF via a 0/1 multiplier
    input (gamma), keeping the program fully static; slots 9..16 are always
    B-chunk, so the PE accumulates them directly in one PSUM bank across
    slots and the A-part is recovered as (accT - accA) at the end.
"""

import sys

sys.path.insert(0, "/opt/trn_rl_repo")

import numpy as np
import concourse.bass as bass
import concourse.mybir as mybir
from concourse import tile
from concourse.bass_utils import run_bass_kernel_spmd
from concourse.vector_clock import ScopedClock


class _LeanTailTileContext(tile.TileContext):
    """TileContext with a single tail barrier: drain + barrier + sem clear.
    The stock exit adds a second all-engine barrier after the sem clear; the
    NEFF's own completion tracking already covers the clears, and dropping it
    saves ~1.5us of tail on every execution."""

    def _drain_and_barrier(self, tick_clock, wait_clock):
        drain_inst = self.nc.sync.drain()
        wait_clock.add_sem_waits(
            drain_inst.ins, ScopedClock({None: tick_clock.global_clock})
        )
        self.nc.all_engine_barrier()
        popped = self.nc._tile_sem_poison_stack.pop()
        assert popped is self._sem_poison
        self.nc.clear_and_free_semaphores(list(self.sems.allocated().values()))

N_CORES = 8
S = 8192
D = 64
CH = 512
NCH = S // CH          # 16 chunks
NSLOT = 17             # (kv, q) pairs per core
JS = 128               # j-subchunk (PSUM partition dim)
NJS = CH // JS         # 4
DA = D + 1             # bias-augmented contraction dim

F32 = mybir.dt.float32
F16 = mybir.dt.float16

# diagonal-slot packed score layout: per j-subchunk s the causal columns are
# i in [128*s, 512); widths (512, 384, 256, 128). s=2 is split 128+128 so
# half0 is exactly [128, 1024] and half1 [128, 256].
# (half, psum col base, pt col base, width, xkv j-subchunk, q col base)
DIAG_MM = [
    (0, 0, 0, 512, 0, 0),
    (0, 512, 512, 384, 1, 128),
    (0, 896, 896, 128, 2, 256),
    (1, 0, 1024, 128, 2, 384),
    (1, 128, 1152, 128, 3, 384),
]
# affine_select head blocks (pt col base, j-subchunk) -- keep i_local >= j
DIAG_SEL = [(0, 0), (512, 1), (896, 2), (1152, 3)]
# z matmuls: (group, pt col base, width, out col base, xkv j-subchunk)
DIAG_Z = [
    (0, 0, 512, 0, 0),
    (0, 512, 384, 128, 1),
    (0, 896, 128, 256, 2),
    (1, 1024, 128, 384, 2),
    (1, 1152, 128, 384, 3),
]


def _split_multiwait(nc, max_waits=1):
    """The walrus build in this container accepts only one sync-wait per
    instruction; hoist extra waits onto preceding same-engine NOPs."""
    for func in nc.m.functions:
        for bb in func.blocks:
            new_insts = []
            for inst in bb.instructions:
                si = inst.sync_info
                if si is not None and si.on_wait and len(si.on_wait) > max_waits:
                    waits = list(si.on_wait)
                    rest, head = waits[:-max_waits], waits[-max_waits:]
                    for j, w in enumerate(rest):
                        nop = mybir.InstNoOp(
                            name=f"{inst.name}-wsplit{j}", ins=[], outs=[]
                        )
                        nop.engine = inst.engine
                        nop.sync_info = mybir.SyncInfo(on_wait=[w], on_update=[])
                        new_insts.append(nop)
                    inst.sync_info = mybir.SyncInfo(
                        on_wait=head, on_update=si.on_update
                    )
                new_insts.append(inst)
            bb.instructions = new_insts


def _schedule(m):
    """Slot list [(kv_block, q_chunk)] for core m. The A diagonal leads (its
    trimmed trapezoid shortens pipeline fill) and the B diagonal trails (its
    short second half shortens the drain into the pair-B finalize)."""
    A, B = m, NCH - 1 - m
    slots = [(A, A)]
    slots += [(b, A) for b in range(A)]
    slots += [(b, B) for b in range(B)]
    slots += [(B, B)]
    gam = [1.0] + [1.0] * A + [0.0] * B + [0.0]
    assert len(slots) == NSLOT
    return slots, gam


def _build_program(repeat=1, dynamic=False, split_multiwait=True):
    nc = bass.Bass()

    # xall = [M^T | wv_aug | slot0 .. slot16]; each slot = [kv rows T+aug |
    # q rows T+aug], all fp16. The weights head rides in slot 0's DMA so the
    # startup critical chain is a single transfer.
    xall_d = nc.declare_dram_parameter(
        "xall", [DA, 2 * DA + 4 * CH + (NSLOT - 2) * 2 * CH], F16, isOutput=False
    )
    # xrows[:, t*260 + s*65 + c] = slot t's kv rows, row-major, augmented
    xrows_d = nc.declare_dram_parameter(
        "xrows", [JS, NSLOT * NJS * DA], F16, isOutput=False
    )
    # gamma (0/1 per slot, broadcast down partitions) fp32
    gpack_d = nc.declare_dram_parameter("gpack", [DA, NSLOT], F32, isOutput=False)
    out_d = nc.declare_dram_parameter("out_pair", [2, CH, D], F32, isOutput=True)

    with _LeanTailTileContext(nc) as tc:
        with (
            tc.tile_pool(name="consts", bufs=1) as consts,
            tc.tile_pool(name="acc", bufs=1) as accp,
            tc.tile_pool(name="warm", bufs=1) as warmp,
            tc.tile_pool(name="slot_in", bufs=5) as slot_in,
            tc.tile_pool(name="rows_in", bufs=5) as rows_in,
            tc.tile_pool(name="qt", bufs=3) as qtp,
            tc.tile_pool(name="pt", bufs=17) as ptp,
            tc.tile_pool(name="fin", bufs=4) as finp,
            tc.tile_pool(name="ps_s", bufs=2, space="PSUM") as ps_s_p,
            tc.tile_pool(name="ps_z", bufs=2, space="PSUM") as ps_z_p,
            tc.tile_pool(name="ps_q", bufs=2, space="PSUM") as ps_q_p,
        ):
            # ---- PE p-state warmup: keep the PE streaming junk while the
            # first DMAs land so real matmuls start at mid/full clock ----
            wtile = warmp.tile([D, 256], F16, tag="w")
            nc.vector.memset(wtile[:], 0.0)
            for _ in range(9):
                ps_w = ps_s_p.tile([16, 256], F32, tag="pss")
                nc.tensor.matmul(
                    ps_w[:], wtile[:, 0:16], wtile[:], start=True, stop=True
                )

            # ---- gamma (schedule metadata) ----
            gam = consts.tile([DA, NSLOT], F32, tag="gam")

            # ---- body (repeat>1 builds a timing-calibration NEFF) ----
            if dynamic and repeat > 1:
                with tc.For_i(0, repeat, 1):
                    _body(nc, tc, slot_in, rows_in, qtp, ptp, finp, accp,
                          ps_s_p, ps_z_p, ps_q_p,
                          xall_d, xrows_d, gpack_d, out_d, gam, wtile)
            else:
                for _rep in range(repeat):
                    _body(nc, tc, slot_in, rows_in, qtp, ptp, finp, accp,
                          ps_s_p, ps_z_p, ps_q_p,
                          xall_d, xrows_d, gpack_d, out_d, gam, wtile)

    if split_multiwait:
        _split_multiwait(nc)
    return nc


def _body(nc, tc, slot_in, rows_in, qtp, ptp, finp, accp,
          ps_s_p, ps_z_p, ps_q_p,
          xall_d, xrows_d, gpack_d, out_d, gam, wtile):
    Exp = mybir.ActivationFunctionType.Exp
    HD = 2 * DA  # weights head width in xall

    xk, xr, qt = {}, {}, {}
    DIAG = (0, NSLOT - 1)

    # slot 0's x rides with the 130-col weights head; [Mt|wv|xq0] lands in
    # the first transfer (all qtil(0) needs), [xkv0|xq1] in the second so
    # qtil(1) never waits on the slot-1 stream
    head = slot_in.tile([DA, HD + 4 * CH], F16, tag="head", bufs=2)
    nc.sync.dma_start(out=head[:, 0:HD + CH], in_=xall_d[:, 0:HD + CH])
    nc.sync.dma_start(
        out=head[:, HD + CH:HD + 4 * CH], in_=xall_d[:, HD + CH:HD + 4 * CH]
    )
    mt = head[:, 0:DA]
    wv = head[:, DA:2 * DA]
    xk[0] = (head[:, HD + CH:HD + 2 * CH], head[:, HD:HD + CH])
    xk[1] = (head[:, HD + 3 * CH:HD + 4 * CH], head[:, HD + 2 * CH:HD + 3 * CH])

    def prep(t):
        if t >= NSLOT:
            return
        if t > 0:
            x_t = slot_in.tile([DA, 2, CH], F16, tag="x_t")
            nc.sync.dma_start(
                out=x_t[:],
                in_=xall_d[:, HD + t * 2 * CH:HD + (t + 1) * 2 * CH],
            )
            xk[t] = (x_t[:, 0, :], x_t[:, 1, :])
        r_t = rows_in.tile([JS, NJS * DA], F16, tag="r_t")
        nc.sync.dma_start(
            out=r_t[:], in_=xrows_d[:, t * NJS * DA:(t + 1) * NJS * DA]
        )
        xr[t] = r_t

    def qtil(t):
        # qtil(t) = M @ xqT(t); PE + DVE round to fp16 for the score matmuls
        if t >= NSLOT:
            return
        ps_q = ps_q_p.tile([DA, CH], F32, tag="psq")
        nc.tensor.matmul(ps_q[:], mt, xk[t][1], start=True, stop=True)
        qh = qtp.tile([DA, CH], F16, tag="qh")
        nc.vector.tensor_copy(qh[:], ps_q[:])
        qt[t] = qh

    def scores_half(t, h):
        pss = ps_s_p.tile([JS, 2 * CH], F32, tag="pss")
        xkv = xk[t][0]
        q = qt[t]
        if t in DIAG:
            for (hh, pb, _ptb, w, s, qb) in DIAG_MM:
                if hh != h:
                    continue
                nc.tensor.matmul(
                    pss[:, pb:pb + w],
                    xkv[:, s * JS:(s + 1) * JS],
                    q[:, qb:qb + w],
                    start=True, stop=True,
                )
        else:
            for hs in range(2):
                s = 2 * h + hs
                nc.tensor.matmul(
                    pss[:, hs * CH:(hs + 1) * CH],
                    xkv[:, s * JS:(s + 1) * JS],
                    q[:],
                    start=True, stop=True,
                )
        return pss

    def exp_half(t, h, pss, pt):
        if t in DIAG and h == 1:
            nc.scalar.activation(pt[:, 1024:1280], pss[:, 0:256], Exp)
        else:
            nc.scalar.activation(
                pt[:, h * 1024:(h + 1) * 1024], pss[:, 0:1024], Exp
            )

    def selects(pt, h):
        # diagonal 128x128 blocks: keep i_local >= j (strict upper zeroed)
        for (ptb, _s) in DIAG_SEL:
            if (ptb >= 1024) != (h == 1):
                continue
            nc.gpsimd.affine_select(
                out=pt[:, ptb:ptb + JS],
                in_=pt[:, ptb:ptb + JS],
                compare_op=mybir.AluOpType.is_ge,
                fill=0.0,
                base=0,
                pattern=[[1, JS]],
                channel_multiplier=-1,
            )

    def z_group(t, grp, pt, ps_z, zstart, zstop):
        rows = xr[t]
        if t in DIAG:
            for (g, ptb, w, ob, s) in DIAG_Z:
                if g != grp:
                    continue
                first = (ptb == 0)
                last = (ptb == 1152)
                nc.tensor.matmul(
                    ps_z[:, ob:ob + w] if not first else ps_z[:, 0:CH],
                    rows[:, s * DA:(s + 1) * DA],
                    pt[:, ptb:ptb + w] if not first else pt[:, 0:CH],
                    start=zstart and first, stop=zstop and last,
                    skip_group_check=True,
                )
        else:
            for hs in range(2):
                s = 2 * grp + hs
                nc.tensor.matmul(
                    ps_z[:],
                    rows[:, s * DA:(s + 1) * DA],
                    pt[:, s * CH:(s + 1) * CH],
                    start=zstart and (s == 0), stop=zstop and (s == 3),
                    skip_group_check=True,
                )

    # ---- accumulators (fp16: they only feed the fp16 finalize matmuls,
    # and the 5e-4 per-add rounding is far inside the error budget) ----
    accT = accp.tile([DA, CH], F16, tag="accT")
    accA = accp.tile([DA, CH], F16, tag="accA")
    bsub = accp.tile([DA, CH], F16, tag="bsub")
    accBh = accp.tile([DA, CH], F16, tag="accBh")

    def accum(t, ps_z):
        # slots 0..8 only; slots 9..16 accumulate in PSUM via the PE itself
        if t == 0:
            nc.vector.tensor_copy(accT[:], ps_z[:])
            nc.vector.tensor_copy(accA[:], ps_z[:])
        else:
            nc.vector.tensor_add(accT[:], accT[:], ps_z[:])
            # accA += gamma_t * z_t (gamma is 0/1 baked per core)
            nc.vector.scalar_tensor_tensor(
                accA[:], ps_z[:], gam[:, t:t + 1], accA[:],
                mybir.AluOpType.mult, mybir.AluOpType.add,
            )

    def fin_step(pair, acch, o, r4, s):
        # out^T subchunk = acch_cols^T @ wv_aug; col 64 = denominator.
        # Every-4th-column interleave: partition p of ps_t holds output row
        # 4p+s, so the DMA's DRAM side is contiguous per partition.
        ps_t = ps_q_p.tile([JS, DA], F32, tag="psq")
        acc4 = acch.rearrange("p (i s) -> p s i", s=NJS)
        nc.tensor.matmul(
            ps_t[:], acc4[:, s, :], wv[:], start=True, stop=True
        )
        nc.vector.reciprocal(r4[:, s:s + 1], ps_t[:, D:DA])
        nc.vector.tensor_scalar_mul(o[:, s, :], ps_t[:, 0:D], r4[:, s:s + 1])
        if s == NJS - 1:
            nc.sync.dma_start(
                out=out_d[pair, :, :].rearrange("(p s) d -> p s d", s=NJS),
                in_=o[:],
            )

    oA = finp.tile([JS, NJS, D], F32, tag="oA")
    oB = finp.tile([JS, NJS, D], F32, tag="oB")
    r4A = finp.tile([JS, NJS], F32, tag="rA")
    r4B = finp.tile([JS, NJS, 1], F32, tag="rB")

    # ---- startup ----
    def prep_xall(t):
        x_t = slot_in.tile([DA, 2, CH], F16, tag="x_t")
        nc.sync.dma_start(
            out=x_t[:], in_=xall_d[:, HD + t * 2 * CH:HD + (t + 1) * 2 * CH]
        )
        xk[t] = (x_t[:, 0, :], x_t[:, 1, :])

    def prep_xrows(t):
        r_t = rows_in.tile([JS, NJS * DA], F16, tag="r_t")
        nc.sync.dma_start(
            out=r_t[:], in_=xrows_d[:, t * NJS * DA:(t + 1) * NJS * DA]
        )
        xr[t] = r_t

    prep_xrows(0)
    prep_xall(2)
    prep_xrows(1)
    prep_xall(3)
    prep_xrows(2)
    nc.sync.dma_start(out=gam[:], in_=gpack_d[:])
    prep_xrows(3)
    qtil(0)
    # two more warmup matmuls keep the PE streak alive across the qcopy(0)
    # wait so the first scores run at full clock
    for _ in range(2):
        ps_w = ps_s_p.tile([16, 256], F32, tag="pss")
        nc.tensor.matmul(
            ps_w[:], wtile[:, 0:16], wtile[:], start=True, stop=True
        )
    qtil(1)
    cur = (scores_half(0, 0), scores_half(0, 1))

    # ---- slot loop (software-pipelined) ----
    pending = None
    accBp = None
    for t in range(NSLOT):
        pt = ptp.tile([JS, 4 * CH], F16, tag="pt")
        exp_half(t, 0, cur[0], pt)
        if t in DIAG:
            selects(pt, 0)
        exp_half(t, 1, cur[1], pt)
        if t in DIAG:
            selects(pt, 1)
        if t < 9:
            ps_z = ps_z_p.tile([DA, CH], F32, tag="psz")
            zstart, zstop = True, True
        else:
            # slots 9..16 are all B-chunk: let the PE accumulate them into
            # one PSUM bank across slots (no per-slot DVE adds needed)
            if accBp is None:
                accBp = ps_z_p.tile([DA, CH], F32, tag="psz")
            ps_z = accBp
            zstart, zstop = (t == 9), (t == NSLOT - 1)
        if t + 1 < NSLOT:
            nh0 = scores_half(t + 1, 0)
        if t in DIAG and t + 1 < NSLOT:
            nh1 = scores_half(t + 1, 1)
        z_group(t, 0, pt, ps_z, zstart, zstop)
        if t + 1 < NSLOT and t not in DIAG:
            nh1 = scores_half(t + 1, 1)
        z_group(t, 1, pt, ps_z, zstart, zstop)
        if t + 1 < NSLOT:
            cur = (nh0, nh1)
        qtil(t + 2)
        prep(t + 4)
        if pending is not None and t <= 9:
            accum(t - 1, pending)
        if t == 9:
            # accA/accT final after accum(8) above; the A-part is removed
            # from the B accumulator at the end (bsub + PSUM B-sum)
            nc.gpsimd.tensor_sub(bsub[:], accT[:], accA[:])
        if 10 <= t <= 13:
            fin_step(0, accA, oA, r4A, t - 10)
        pending = ps_z

    nc.vector.tensor_add(accBh[:], bsub[:], accBp[:])
    ps_t4 = ps_s_p.tile([JS, NJS, DA], F32, tag="pss")
    accB4 = accBh.rearrange("p (i s) -> p s i", s=NJS)
    for s in range(NJS):
        nc.tensor.matmul(
            ps_t4[:, s, :], accB4[:, s, :], wv[:], start=True, stop=True
        )
    nc.vector.reciprocal(r4B[:], ps_t4[:, :, D:DA])
    nc.vector.tensor_mul(
        oB[:], ps_t4[:, :, 0:D], r4B.broadcast_to([JS, NJS, D])
    )
    nc.sync.dma_start(
        out=out_d[1, :, :].rearrange("(p s) d -> p s d", s=NJS), in_=oB[:]
    )


_NC_CACHE = None


def _get_program():
    global _NC_CACHE
    if _NC_CACHE is None:
        _NC_CACHE = _build_program()
    return _NC_CACHE


def _host_inputs(x, w_q, b_q, w_k, b_k, w_v, b_v):
    """Per-core input dicts. Host work is layout only: transpose / gather /
    concat of x rows, fp16 conversion, weight reshuffles, constant tables."""
    x = np.ascontiguousarray(np.asarray(x, dtype=np.float32))
    scale = 1.0 / np.sqrt(np.float32(D))

    wk_aug = np.concatenate([np.asarray(w_k, np.float32).T,
                             np.asarray(b_k, np.float32)[None, :]], axis=0)
    wq_aug = np.concatenate([np.asarray(w_q, np.float32).T,
                             np.asarray(b_q, np.float32)[None, :]], axis=0) * scale
    wv_aug = np.zeros((DA, DA), np.float32)
    wv_aug[:D, :D] = np.asarray(w_v, np.float32).T
    wv_aug[D, :D] = np.asarray(b_v, np.float32)
    wv_aug[D, D] = 1.0
    m_mat = wk_aug @ wq_aug.T                     # [65, 65]

    xT_aug = np.empty((DA, S), np.float16)
    xT_aug[:D] = x.T
    xT_aug[D] = 1.0
    x_aug = np.empty((S, DA), np.float16)
    x_aug[:, :D] = x
    x_aug[:, D] = 1.0

    in_maps = []
    for m in range(N_CORES):
        slots, gam = _schedule(m)
        xall = np.empty((DA, 2 * DA + 4 * CH + (NSLOT - 2) * 2 * CH), np.float16)
        xall[:, 0:DA] = m_mat.T
        xall[:, DA:2 * DA] = wv_aug
        hb = 2 * DA
        # head x: [xq0 | xkv0 | xq1 | xkv1]
        for i, (b, c) in enumerate(slots[:2]):
            xall[:, hb + 2 * i * CH:hb + (2 * i + 1) * CH] = \
                xT_aug[:, c * CH:(c + 1) * CH]
            xall[:, hb + (2 * i + 1) * CH:hb + (2 * i + 2) * CH] = \
                xT_aug[:, b * CH:(b + 1) * CH]
        xs = xall[:, hb + 4 * CH:].reshape(DA, NSLOT - 2, 2, CH)
        xrows = np.empty((JS, NSLOT, NJS, DA), np.float16)
        for t, (b, c) in enumerate(slots):
            if t >= 2:
                xs[:, t - 2, 0, :] = xT_aug[:, b * CH:(b + 1) * CH]
                xs[:, t - 2, 1, :] = xT_aug[:, c * CH:(c + 1) * CH]
            blk = x_aug[b * CH:(b + 1) * CH]      # [512, 65]
            xrows[:, t] = blk.reshape(NJS, JS, DA).transpose(1, 0, 2)
        gpack = np.broadcast_to(
            np.asarray(gam, np.float32)[None, :], (DA, NSLOT)
        ).copy()
        in_maps.append({
            "xall": xall,
            "xrows": xrows.reshape(JS, NSLOT * NJS * DA),
            "gpack": gpack,
        })
    return in_maps


def _assemble(results):
    out = np.empty((S, D), np.float32)
    for m in range(N_CORES):
        op = results[m]["out_pair"]
        A, B = m, NCH - 1 - m
        out[A * CH:(A + 1) * CH] = op[0]
        out[B * CH:(B + 1) * CH] = op[1]
    return out


def kernel(x, w_q, b_q, w_k, b_k, w_v, b_v):
    nc = _get_program()
    in_maps = _host_inputs(x, w_q, b_q, w_k, b_k, w_v, b_v)
    res = run_bass_kernel_spmd(nc, in_maps, list(range(N_CORES)))
    return _assemble(res.results)


# revision 34
# speedup vs baseline: 1.2472x; 1.2472x over previous
"""Causal single-head attention (S=8192, d=64) on 8 Trainium2 NeuronCores.

Strategy (sequence-parallel, load-balanced over the causal triangle):
  - Split the sequence into 16 chunks of 512 rows. Core m owns query chunks
    A=m and B=15-m, so every core sees (m+1) + (16-m) = 17 (kv-block, q-chunk)
    pairs of 512x512 scores -- perfectly balanced.
  - Each pair is one "slot". The SPMD program is identical on all cores; the
    per-core schedule is baked into the *input data* (host gathers the slot's
    kv rows and q rows, transposed and bias-augmented, in fp16).
  - Per slot: qtil = M @ xqT (M = wk_aug wq_aug^T / 8, host-precomputed from
    weights), scores sT[j, i] = xkvT^T @ qtil (kv j on partitions), pT =
    exp(sT).
  - The V projection is reassociated out of the slot loop entirely:
      P @ (Xk wv^T + bv) = wv_aug^T @ (Xk_aug^T @ P^T)
    so each slot only computes z = x_rows^T @ pT (x_rows = the kv rows in
    row-major, bias-augmented; row 64 of z = softmax denominator), and wv_aug
    is applied once per q chunk at the end, fused into the PE transpose that
    produces the output layout.
  - The causal mask on the two diagonal slots is handled by computing only the
    causal trapezoid (packed into PSUM) + post-exp affine_select on the four
    128x128 diagonal blocks, exactly matching softmax(scores - 1e10*mask).
  - Slots are ordered [diag-A, A-pairs, B-pairs, diag-B]. Slots 0..8 (the
    data-dependent A/B mix) accumulate z into SBUF via a 0/1 multiplier
    input (gamma), keeping the program fully static; slots 9..16 are always
    B-chunk, so the PE accumulates them directly in one PSUM bank across
    slots and the A-part is recovered as (accT - accA) at the end.
"""

import sys

sys.path.insert(0, "/opt/trn_rl_repo")

import numpy as np
import concourse.bass as bass
import concourse.mybir as mybir
from concourse import tile
from concourse.bass_utils import run_bass_kernel_spmd
from concourse.vector_clock import ScopedClock


class _LeanTailTileContext(tile.TileContext):
    """TileContext with a single tail barrier: drain + barrier + sem clear.
    The stock exit adds a second all-engine barrier after the sem clear; the
    NEFF's own completion tracking already covers the clears, and dropping it
    saves ~1.5us of tail on every execution."""

    def _drain_and_barrier(self, tick_clock, wait_clock):
        drain_inst = self.nc.sync.drain()
        wait_clock.add_sem_waits(
            drain_inst.ins, ScopedClock({None: tick_clock.global_clock})
        )
        self.nc.all_engine_barrier()
        popped = self.nc._tile_sem_poison_stack.pop()
        assert popped is self._sem_poison
        self.nc.clear_and_free_semaphores(list(self.sems.allocated().values()))

N_CORES = 8
S = 8192
D = 64
CH = 512
NCH = S // CH          # 16 chunks
NSLOT = 17             # (kv, q) pairs per core
JS = 128               # j-subchunk (PSUM partition dim)
NJS = CH // JS         # 4
DA = D + 1             # bias-augmented contraction dim

F32 = mybir.dt.float32
F16 = mybir.dt.float16

# diagonal-slot packed score layout: per j-subchunk s the causal columns are
# i in [128*s, 512); widths (512, 384, 256, 128). s=2 is split 128+128 so
# half0 is exactly [128, 1024] and half1 [128, 256].
# (half, psum col base, pt col base, width, xkv j-subchunk, q col base)
DIAG_MM = [
    (0, 0, 0, 512, 0, 0),
    (0, 512, 512, 384, 1, 128),
    (0, 896, 896, 128, 2, 256),
    (1, 0, 1024, 128, 2, 384),
    (1, 128, 1152, 128, 3, 384),
]
# affine_select head blocks (pt col base, j-subchunk) -- keep i_local >= j
DIAG_SEL = [(0, 0), (512, 1), (896, 2), (1152, 3)]
# z matmuls: (group, pt col base, width, out col base, xkv j-subchunk)
DIAG_Z = [
    (0, 0, 512, 0, 0),
    (0, 512, 384, 128, 1),
    (0, 896, 128, 256, 2),
    (1, 1024, 128, 384, 2),
    (1, 1152, 128, 384, 3),
]


def _split_multiwait(nc, max_waits=1):
    """The walrus build in this container accepts only one sync-wait per
    instruction; hoist extra waits onto preceding same-engine NOPs."""
    for func in nc.m.functions:
        for bb in func.blocks:
            new_insts = []
            for inst in bb.instructions:
                si = inst.sync_info
                if si is not None and si.on_wait and len(si.on_wait) > max_waits:
                    waits = list(si.on_wait)
                    rest, head = waits[:-max_waits], waits[-max_waits:]
                    for j, w in enumerate(rest):
                        nop = mybir.InstNoOp(
                            name=f"{inst.name}-wsplit{j}", ins=[], outs=[]
                        )
                        nop.engine = inst.engine
                        nop.sync_info = mybir.SyncInfo(on_wait=[w], on_update=[])
                        new_insts.append(nop)
                    inst.sync_info = mybir.SyncInfo(
                        on_wait=head, on_update=si.on_update
                    )
                new_insts.append(inst)
            bb.instructions = new_insts


def _schedule(m):
    """Slot list [(kv_block, q_chunk)] for core m. The A diagonal leads (its
    trimmed trapezoid shortens pipeline fill) and the B diagonal trails (its
    short second half shortens the drain into the pair-B finalize)."""
    A, B = m, NCH - 1 - m
    slots = [(A, A)]
    slots += [(b, A) for b in range(A)]
    slots += [(b, B) for b in range(B)]
    slots += [(B, B)]
    gam = [1.0] + [1.0] * A + [0.0] * B + [0.0]
    assert len(slots) == NSLOT
    return slots, gam


def _build_program(repeat=1, dynamic=False, split_multiwait=True):
    nc = bass.Bass()

    # xall = [M^T | wv_aug | slot0 .. slot16]; each slot = [kv rows T+aug |
    # q rows T+aug], all fp16. The weights head rides in slot 0's DMA so the
    # startup critical chain is a single transfer.
    xall_d = nc.declare_dram_parameter(
        "xall", [DA, 2 * DA + 4 * CH + (NSLOT - 2) * 2 * CH], F16, isOutput=False
    )
    # xrows[:, t*260 + s*65 + c] = slot t's kv rows, row-major, augmented
    xrows_d = nc.declare_dram_parameter(
        "xrows", [JS, NSLOT * NJS * DA], F16, isOutput=False
    )
    # gamma (0/1 per slot, broadcast down partitions) fp32
    gpack_d = nc.declare_dram_parameter("gpack", [DA, NSLOT], F32, isOutput=False)
    out_d = nc.declare_dram_parameter("out_pair", [2, CH, D], F32, isOutput=True)

    with _LeanTailTileContext(nc) as tc:
        with (
            tc.tile_pool(name="consts", bufs=1) as consts,
            tc.tile_pool(name="acc", bufs=1) as accp,
            tc.tile_pool(name="warm", bufs=1) as warmp,
            tc.tile_pool(name="slot_in", bufs=5) as slot_in,
            tc.tile_pool(name="rows_in", bufs=5) as rows_in,
            tc.tile_pool(name="qt", bufs=3) as qtp,
            tc.tile_pool(name="pt", bufs=17) as ptp,
            tc.tile_pool(name="fin", bufs=4) as finp,
            tc.tile_pool(name="ps_s", bufs=2, space="PSUM") as ps_s_p,
            tc.tile_pool(name="ps_z", bufs=2, space="PSUM") as ps_z_p,
            tc.tile_pool(name="ps_q", bufs=2, space="PSUM") as ps_q_p,
        ):
            # ---- PE p-state warmup: keep the PE streaming junk while the
            # first DMAs land so real matmuls start at mid/full clock ----
            wtile = warmp.tile([D, 256], F16, tag="w")
            nc.vector.memset(wtile[:], 0.0)
            for _ in range(9):
                ps_w = ps_s_p.tile([16, 256], F32, tag="pss")
                nc.tensor.matmul(
                    ps_w[:], wtile[:, 0:16], wtile[:], start=True, stop=True
                )

            # ---- gamma (schedule metadata) ----
            gam = consts.tile([DA, NSLOT], F32, tag="gam")

            # ---- body (repeat>1 builds a timing-calibration NEFF) ----
            if dynamic and repeat > 1:
                with tc.For_i(0, repeat, 1):
                    _body(nc, tc, slot_in, rows_in, qtp, ptp, finp, accp,
                          ps_s_p, ps_z_p, ps_q_p,
                          xall_d, xrows_d, gpack_d, out_d, gam, wtile)
            else:
                for _rep in range(repeat):
                    _body(nc, tc, slot_in, rows_in, qtp, ptp, finp, accp,
                          ps_s_p, ps_z_p, ps_q_p,
                          xall_d, xrows_d, gpack_d, out_d, gam, wtile)

    if split_multiwait:
        _split_multiwait(nc)
    return nc


def _body(nc, tc, slot_in, rows_in, qtp, ptp, finp, accp,
          ps_s_p, ps_z_p, ps_q_p,
          xall_d, xrows_d, gpack_d, out_d, gam, wtile):
    Exp = mybir.ActivationFunctionType.Exp
    HD = 2 * DA  # weights head width in xall

    xk, xr, qt = {}, {}, {}
    DIAG = (0, NSLOT - 1)

    # slot 0's x rides with the 130-col weights head; [Mt|wv|xq0] lands in
    # the first transfer (all qtil(0) needs), [xkv0|xq1] in the second so
    # qtil(1) never waits on the slot-1 stream
    head = slot_in.tile([DA, HD + 4 * CH], F16, tag="head", bufs=2)
    nc.sync.dma_start(out=head[:, 0:HD + CH], in_=xall_d[:, 0:HD + CH])
    nc.sync.dma_start(
        out=head[:, HD + CH:HD + 4 * CH], in_=xall_d[:, HD + CH:HD + 4 * CH]
    )
    mt = head[:, 0:DA]
    wv = head[:, DA:2 * DA]
    xk[0] = (head[:, HD + CH:HD + 2 * CH], head[:, HD:HD + CH])
    xk[1] = (head[:, HD + 3 * CH:HD + 4 * CH], head[:, HD + 2 * CH:HD + 3 * CH])

    def prep(t):
        if t >= NSLOT:
            return
        if t > 0:
            x_t = slot_in.tile([DA, 2, CH], F16, tag="x_t")
            nc.sync.dma_start(
                out=x_t[:],
                in_=xall_d[:, HD + t * 2 * CH:HD + (t + 1) * 2 * CH],
            )
            xk[t] = (x_t[:, 0, :], x_t[:, 1, :])
        r_t = rows_in.tile([JS, NJS * DA], F16, tag="r_t")
        nc.sync.dma_start(
            out=r_t[:], in_=xrows_d[:, t * NJS * DA:(t + 1) * NJS * DA]
        )
        xr[t] = r_t

    def qtil(t):
        # qtil(t) = M @ xqT(t); PE + DVE round to fp16 for the score matmuls
        if t >= NSLOT:
            return
        ps_q = ps_q_p.tile([DA, CH], F32, tag="psq")
        nc.tensor.matmul(ps_q[:], mt, xk[t][1], start=True, stop=True)
        qh = qtp.tile([DA, CH], F16, tag="qh")
        nc.vector.tensor_copy(qh[:], ps_q[:])
        qt[t] = qh

    def scores_half(t, h):
        if t == 1 and h == 0:
            # pipeline fill: slot 1's first score half goes through two
            # spare 1-bank psq-tag tiles so exp(1)h0 need not wait for
            # exp(0)h0 to release the double-buffered score pool
            xkv = xk[1][0]
            q = qt[1]
            tiles = []
            for s in range(2):
                pf = ps_q_p.tile([JS, CH], F32, tag="psq")
                nc.tensor.matmul(
                    pf[:], xkv[:, s * JS:(s + 1) * JS], q[:],
                    start=True, stop=True,
                )
                tiles.append(pf)
            return tuple(tiles)
        pss = ps_s_p.tile([JS, 2 * CH], F32, tag="pss")
        xkv = xk[t][0]
        q = qt[t]
        if t in DIAG:
            for (hh, pb, _ptb, w, s, qb) in DIAG_MM:
                if hh != h:
                    continue
                nc.tensor.matmul(
                    pss[:, pb:pb + w],
                    xkv[:, s * JS:(s + 1) * JS],
                    q[:, qb:qb + w],
                    start=True, stop=True,
                )
        else:
            for hs in range(2):
                s = 2 * h + hs
                nc.tensor.matmul(
                    pss[:, hs * CH:(hs + 1) * CH],
                    xkv[:, s * JS:(s + 1) * JS],
                    q[:],
                    start=True, stop=True,
                )
        return pss

    def exp_half(t, h, pss, pt):
        if isinstance(pss, tuple):
            for s, pf in enumerate(pss):
                nc.scalar.activation(
                    pt[:, s * CH:(s + 1) * CH], pf[:], Exp
                )
        elif t in DIAG and h == 1:
            nc.scalar.activation(pt[:, 1024:1280], pss[:, 0:256], Exp)
        else:
            nc.scalar.activation(
                pt[:, h * 1024:(h + 1) * 1024], pss[:, 0:1024], Exp
            )

    def selects(pt, h):
        # diagonal 128x128 blocks: keep i_local >= j (strict upper zeroed)
        for (ptb, _s) in DIAG_SEL:
            if (ptb >= 1024) != (h == 1):
                continue
            nc.gpsimd.affine_select(
                out=pt[:, ptb:ptb + JS],
                in_=pt[:, ptb:ptb + JS],
                compare_op=mybir.AluOpType.is_ge,
                fill=0.0,
                base=0,
                pattern=[[1, JS]],
                channel_multiplier=-1,
            )

    HCH = CH // 2

    def z_group(t, grp, pt, ps_z, zstart, zstop):
        rows = xr[t]
        if t in DIAG:
            for (g, ptb, w, ob, s) in DIAG_Z:
                if g != grp:
                    continue
                first = (ptb == 0)
                last = (ptb == 1152)
                nc.tensor.matmul(
                    ps_z[:, ob:ob + w] if not first else ps_z[:, 0:CH],
                    rows[:, s * DA:(s + 1) * DA],
                    pt[:, ptb:ptb + w] if not first else pt[:, 0:CH],
                    start=zstart and first, stop=zstop and last,
                    skip_group_check=True,
                )
        else:
            for hs in range(2):
                s = 2 * grp + hs
                nc.tensor.matmul(
                    ps_z[:],
                    rows[:, s * DA:(s + 1) * DA],
                    pt[:, s * CH:(s + 1) * CH],
                    start=zstart and (s == 0), stop=zstop and (s == 3),
                    skip_group_check=True,
                )

    # ---- accumulators (fp16: they only feed the fp16 finalize matmuls,
    # and the 5e-4 per-add rounding is far inside the error budget) ----
    accT = accp.tile([DA, CH], F16, tag="accT")
    accA = accp.tile([DA, CH], F16, tag="accA")
    bsub = accp.tile([DA, CH], F16, tag="bsub")
    accBh = accp.tile([DA, CH], F16, tag="accBh")

    def accum(t, ps_z):
        # slots 0..8 only; slots 9..16 accumulate in PSUM via the PE itself
        if t == 0:
            nc.vector.tensor_copy(accT[:], ps_z[:])
            nc.vector.tensor_copy(accA[:], ps_z[:])
        else:
            nc.vector.tensor_add(accT[:], accT[:], ps_z[:])
            # accA += gamma_t * z_t (gamma is 0/1 baked per core)
            nc.vector.scalar_tensor_tensor(
                accA[:], ps_z[:], gam[:, t:t + 1], accA[:],
                mybir.AluOpType.mult, mybir.AluOpType.add,
            )

    def fin_step(pair, acch, o, r4, s):
        # out^T subchunk = acch_cols^T @ wv_aug; col 64 = denominator.
        # Every-4th-column interleave: partition p of ps_t holds output row
        # 4p+s, so the DMA's DRAM side is contiguous per partition.
        ps_t = ps_q_p.tile([JS, DA], F32, tag="psq")
        acc4 = acch.rearrange("p (i s) -> p s i", s=NJS)
        nc.tensor.matmul(
            ps_t[:], acc4[:, s, :], wv[:], start=True, stop=True
        )
        nc.vector.reciprocal(r4[:, s:s + 1], ps_t[:, D:DA])
        nc.vector.tensor_scalar_mul(o[:, s, :], ps_t[:, 0:D], r4[:, s:s + 1])
        if s == NJS - 1:
            nc.sync.dma_start(
                out=out_d[pair, :, :].rearrange("(p s) d -> p s d", s=NJS),
                in_=o[:],
            )

    oA = finp.tile([JS, NJS, D], F32, tag="oA")
    oB = finp.tile([JS, NJS, D], F32, tag="oB")
    r4A = finp.tile([JS, NJS], F32, tag="rA")
    r4B = finp.tile([JS, NJS, 1], F32, tag="rB")

    # ---- startup ----
    def prep_xall(t):
        x_t = slot_in.tile([DA, 2, CH], F16, tag="x_t")
        nc.sync.dma_start(
            out=x_t[:], in_=xall_d[:, HD + t * 2 * CH:HD + (t + 1) * 2 * CH]
        )
        xk[t] = (x_t[:, 0, :], x_t[:, 1, :])

    def prep_xrows(t):
        r_t = rows_in.tile([JS, NJS * DA], F16, tag="r_t")
        nc.sync.dma_start(
            out=r_t[:], in_=xrows_d[:, t * NJS * DA:(t + 1) * NJS * DA]
        )
        xr[t] = r_t

    prep_xrows(0)
    prep_xall(2)
    prep_xrows(1)
    prep_xall(3)
    prep_xrows(2)
    nc.sync.dma_start(out=gam[:], in_=gpack_d[:])
    prep_xrows(3)
    qtil(0)
    # two more warmup matmuls keep the PE streak alive across the qcopy(0)
    # wait so the first scores run at full clock
    for _ in range(2):
        ps_w = ps_s_p.tile([16, 256], F32, tag="pss")
        nc.tensor.matmul(
            ps_w[:], wtile[:, 0:16], wtile[:], start=True, stop=True
        )
    qtil(1)
    cur = (scores_half(0, 0), scores_half(0, 1))

    # ---- slot loop (software-pipelined) ----
    pending = None
    accBp = None
    for t in range(NSLOT):
        pt = ptp.tile([JS, 4 * CH], F16, tag="pt")
        exp_half(t, 0, cur[0], pt)
        if t in DIAG:
            selects(pt, 0)
        exp_half(t, 1, cur[1], pt)
        if t in DIAG:
            selects(pt, 1)
        if t < 9:
            ps_z = ps_z_p.tile([DA, CH], F32, tag="psz")
            zstart, zstop = True, True
        else:
            # slots 9..16 are all B-chunk: let the PE accumulate them into
            # one PSUM bank across slots (no per-slot DVE adds needed)
            if accBp is None:
                accBp = ps_z_p.tile([DA, CH], F32, tag="psz")
            ps_z = accBp
            zstart, zstop = (t == 9), (t == NSLOT - 1)
        if t + 1 < NSLOT:
            nh0 = scores_half(t + 1, 0)
        if t in DIAG and t + 1 < NSLOT:
            nh1 = scores_half(t + 1, 1)
        z_group(t, 0, pt, ps_z, zstart, zstop)
        if t + 1 < NSLOT and t not in DIAG:
            nh1 = scores_half(t + 1, 1)
        z_group(t, 1, pt, ps_z, zstart, zstop)
        if t + 1 < NSLOT:
            cur = (nh0, nh1)
        qtil(t + 2)
        prep(t + 4)
        if pending is not None and t <= 9:
            accum(t - 1, pending)
        if t == 9:
            # accA/accT final after accum(8) above; the A-part is removed
            # from the B accumulator at the end (bsub + PSUM B-sum)
            nc.gpsimd.tensor_sub(bsub[:], accT[:], accA[:])
        if 10 <= t <= 13:
            fin_step(0, accA, oA, r4A, t - 10)
        pending = ps_z

    nc.vector.tensor_add(accBh[:], bsub[:], accBp[:])
    ps_t4 = ps_s_p.tile([JS, NJS, DA], F32, tag="pss")
    accB4 = accBh.rearrange("p (i s) -> p s i", s=NJS)
    for s in range(NJS):
        nc.tensor.matmul(
            ps_t4[:, s, :], accB4[:, s, :], wv[:], start=True, stop=True
        )
    nc.vector.reciprocal(r4B[:], ps_t4[:, :, D:DA])
    nc.vector.tensor_mul(
        oB[:], ps_t4[:, :, 0:D], r4B.broadcast_to([JS, NJS, D])
    )
    nc.sync.dma_start(
        out=out_d[1, :, :].rearrange("(p s) d -> p s d", s=NJS), in_=oB[:]
    )


_NC_CACHE = None


def _get_program():
    global _NC_CACHE
    if _NC_CACHE is None:
        _NC_CACHE = _build_program()
    return _NC_CACHE


def _host_inputs(x, w_q, b_q, w_k, b_k, w_v, b_v):
    """Per-core input dicts. Host work is layout only: transpose / gather /
    concat of x rows, fp16 conversion, weight reshuffles, constant tables."""
    x = np.ascontiguousarray(np.asarray(x, dtype=np.float32))
    scale = 1.0 / np.sqrt(np.float32(D))

    wk_aug = np.concatenate([np.asarray(w_k, np.float32).T,
                             np.asarray(b_k, np.float32)[None, :]], axis=0)
    wq_aug = np.concatenate([np.asarray(w_q, np.float32).T,
                             np.asarray(b_q, np.float32)[None, :]], axis=0) * scale
    wv_aug = np.zeros((DA, DA), np.float32)
    wv_aug[:D, :D] = np.asarray(w_v, np.float32).T
    wv_aug[D, :D] = np.asarray(b_v, np.float32)
    wv_aug[D, D] = 1.0
    m_mat = wk_aug @ wq_aug.T                     # [65, 65]

    xT_aug = np.empty((DA, S), np.float16)
    xT_aug[:D] = x.T
    xT_aug[D] = 1.0
    x_aug = np.empty((S, DA), np.float16)
    x_aug[:, :D] = x
    x_aug[:, D] = 1.0

    in_maps = []
    for m in range(N_CORES):
        slots, gam = _schedule(m)
        xall = np.empty((DA, 2 * DA + 4 * CH + (NSLOT - 2) * 2 * CH), np.float16)
        xall[:, 0:DA] = m_mat.T
        xall[:, DA:2 * DA] = wv_aug
        hb = 2 * DA
        # head x: [xq0 | xkv0 | xq1 | xkv1]
        for i, (b, c) in enumerate(slots[:2]):
            xall[:, hb + 2 * i * CH:hb + (2 * i + 1) * CH] = \
                xT_aug[:, c * CH:(c + 1) * CH]
            xall[:, hb + (2 * i + 1) * CH:hb + (2 * i + 2) * CH] = \
                xT_aug[:, b * CH:(b + 1) * CH]
        xs = xall[:, hb + 4 * CH:].reshape(DA, NSLOT - 2, 2, CH)
        xrows = np.empty((JS, NSLOT, NJS, DA), np.float16)
        for t, (b, c) in enumerate(slots):
            if t >= 2:
                xs[:, t - 2, 0, :] = xT_aug[:, b * CH:(b + 1) * CH]
                xs[:, t - 2, 1, :] = xT_aug[:, c * CH:(c + 1) * CH]
            blk = x_aug[b * CH:(b + 1) * CH]      # [512, 65]
            xrows[:, t] = blk.reshape(NJS, JS, DA).transpose(1, 0, 2)
        gpack = np.broadcast_to(
            np.asarray(gam, np.float32)[None, :], (DA, NSLOT)
        ).copy()
        in_maps.append({
            "xall": xall,
            "xrows": xrows.reshape(JS, NSLOT * NJS * DA),
            "gpack": gpack,
        })
    return in_maps


def _assemble(results):
    out = np.empty((S, D), np.float32)
    for m in range(N_CORES):
        op = results[m]["out_pair"]
        A, B = m, NCH - 1 - m
        out[A * CH:(A + 1) * CH] = op[0]
        out[B * CH:(B + 1) * CH] = op[1]
    return out


def kernel(x, w_q, b_q, w_k, b_k, w_v, b_v):
    nc = _get_program()
    in_maps = _host_inputs(x, w_q, b_q, w_k, b_k, w_v, b_v)
    res = run_bass_kernel_spmd(nc, in_maps, list(range(N_CORES)))
    return _assemble(res.results)
